# revision 1
# baseline (speedup 1.0000x reference)
"""Trainium2 Bass kernel for CrossAttention (B=2, Nq=Nkv=2048, Cq=1024, Ck=768, H=8, D=64).

Sharding: batch*heads across 8 cores — core c handles batch c//4 and heads
{2*(c%4), 2*(c%4)+1} (a 128-wide slice of the inner dim I=512).  Wq/Wk/Wv are
column-sharded, Wo row-sharded; each core produces a partial (2048, 1024)
output and the host sums the 4 partials per batch (the "all-reduce").

Per-core dataflow (all matmuls in float32r = full-rate fp32 on the PE):
  xT (Cq x Nq) -> QT = Wq^T @ x       (128 x Nq)   [I_c on partitions]
  cT (Ck x Nkv), streamed in 512-key blocks:
      KT block = Wk^T @ ctx, VT block = Wv^T @ ctx, then PE-transpose VT
      into V j-tiles (the [V_h | 1] "vaug" layout).  Block production is
      software-pipelined into the first superchunk's attention loop.
  scores  S^T[j,i] = K_h Q_h^T  (transposed: j on partitions, 512-wide i)
  E = exp(SCALE * S^T)  via ScalarE straight out of PSUM (1024-wide reads)
  O^T[d,i] accumulated as matmul(lhsT=[V_h | ones], rhs=E) over j-tiles;
     the ones column yields the softmax denominators in the same pass.
  normalize O^T columns by 1/denom (DRAM-roundtrip row broadcast + DVE mul;
     head 1 partition-shifted into rows 64..127 via SBUF-SBUF DMA)
  out_partial = O @ Wo_slice, pipelined into the next superchunk's loop.
"""

import numpy as np
from contextlib import ExitStack

import concourse.bass as bass
import concourse.bacc as bacc
import concourse.mybir as mybir
import concourse.tile as tile
from concourse.masks import make_identity

F32 = mybir.dt.float32
FR = mybir.dt.float32r     # full-rate fp32 matmul mode on trn2
EXP = mybir.ActivationFunctionType.Exp

B, NQ, NKV, CQ, CK, H, D = 2, 2048, 2048, 1024, 768, 8, 64
HPC = 2                 # heads per core
IC = HPC * D            # 128: per-core slice of I = H*D
N_CORES = 8
SCALE = float(D) ** -0.5


def build_nc(nq=NQ, nkv=NKV, cq=CQ, ck=CK, num_devices=N_CORES, reps=1, loop_iters=1):
    """Build + compile the per-core SPMD Bass program."""
    assert nq % 1024 == 0 and nkv % 512 == 0 and cq % 512 == 0 and ck % 128 == 0
    KQ = cq // 128          # contraction tiles for the q projection
    KK = ck // 128          # contraction tiles for k/v projections
    NJT = nkv // 128        # key tiles
    NJB = nkv // 512        # key blocks (4 j-tiles each)
    NICP = nq // 1024       # 1024-wide query superchunks
    NCC = cq // 512         # output-column chunks

    nc = bacc.Bacc("TRN2", target_bir_lowering=False, debug=False,
                   enable_asserts=False, num_devices=num_devices)

    xT = nc.dram_tensor("xT", [cq, nq], FR, kind="ExternalInput").ap()
    cT = nc.dram_tensor("cT", [ck, nkv], FR, kind="ExternalInput").ap()
    wq = nc.dram_tensor("wq", [128, KQ, 128], FR, kind="ExternalInput").ap()
    wk = nc.dram_tensor("wk", [128, KK, 128], FR, kind="ExternalInput").ap()
    wv = nc.dram_tensor("wv", [128, KK, 128], FR, kind="ExternalInput").ap()
    wo = nc.dram_tensor("wo", [128, cq], FR, kind="ExternalInput").ap()
    out_p = nc.dram_tensor("out_p", [nq, cq], F32, kind="ExternalOutput").ap()

    with tile.TileContext(nc) as tc, ExitStack() as ctx:
        const = ctx.enter_context(tc.tile_pool(name="const", bufs=1))
        wq_sb = const.tile([128, KQ, 128], FR)
        wk_sb = const.tile([128, KK, 128], FR)
        wv_sb = const.tile([128, KK, 128], FR)
        wo_sb = const.tile([128, cq], FR)
        ident_f = const.tile([128, 128], F32)
        ident = const.tile([128, 128], FR)
        ones_f = const.tile([128, 4], F32)

        persist = ctx.enter_context(tc.tile_pool(name="persist", bufs=1))
        qt_sb = persist.tile([128, nq], FR)       # Q^T, I_c x Nq
        kt_jbs = [persist.tile([128, 512], FR, name=f"kt{jb}")
                  for jb in range(NJB)]           # K^T per key block
        vaug_jbs = [persist.tile([128, 4, 130], FR, name=f"vg{jb}")
                    for jb in range(NJB)]         # [V_h0 |1| V_h1 |1] per j-tile
        ot_sb = persist.tile([128, nq], FR)       # normalized O^T

        # PSUM pools: "s" = 2 x (128,1024) slots shared by scores / projections /
        # phase-A psum; "ot" = 4 x 1-bank accumulator slots.
        s_pool = ctx.enter_context(tc.tile_pool(name="s", bufs=2, space="PSUM"))
        ot_pool = ctx.enter_context(tc.tile_pool(name="otp", bufs=4, space="PSUM"))
        e_pool = ctx.enter_context(tc.tile_pool(name="e", bufs=3))
        r_pool = ctx.enter_context(tc.tile_pool(name="r", bufs=1))
        o_pool = ctx.enter_context(tc.tile_pool(name="o", bufs=3))
        d_pool = ctx.enter_context(tc.tile_pool(name="d", bufs=4, space="DRAM"))
        xq_pool = ctx.enter_context(tc.tile_pool(name="xq", bufs=1))
        ck_pool = ctx.enter_context(tc.tile_pool(name="ck", bufs=2))

        with tc.high_priority():
            nc.sync.dma_start(wq_sb[:], wq)
            nc.sync.dma_start(wk_sb[:], wk)
            nc.sync.dma_start(wv_sb[:], wv)
            nc.sync.dma_start(wo_sb[:], wo)
        make_identity(nc, ident_f[:])
        nc.vector.tensor_copy(ident[:], ident_f[:])
        # softmax-denominator ones columns (static, written once)
        nc.vector.memset(ones_f[:], 1.0)
        for jb in range(NJB):
            nc.vector.tensor_copy(vaug_jbs[jb][:, 0:4, 64:65], ones_f[:, 0:4])
            nc.vector.tensor_copy(vaug_jbs[jb][:, 0:4, 129:130], ones_f[:, 0:4])

        def bcast64(row_ap, dst_ap, nm):
            """Broadcast a 1-partition 512-wide row to 64 partitions via DRAM."""
            scr = d_pool.tile([512], F32, tag="scr", name=f"scr_{nm}")
            nc.sync.dma_start(scr[:], row_ap)
            bsrc = bass.AP(scr.tensor, scr.offset, [[0, 64], [1, 512]])
            nc.sync.dma_start(dst_ap, bsrc)

        def proj_tile(gi, tail):
            """Project one 128-query tile of normalized O^T through Wo."""
            i0 = gi * 128
            o_sb = o_pool.tile([128, cq], F32, tag="o", name="o_sb")
            for n2 in range(NCC):
                pp = s_pool.tile([128, 512], F32, tag="s", name="pp")
                nc.tensor.matmul(pp[:], ot_sb[:, i0:i0 + 128],
                                 wo_sb[:, n2 * 512:(n2 + 1) * 512],
                                 start=True, stop=True)
                if tail and n2 % 2 == 1:
                    nc.scalar.copy(o_sb[:, n2 * 512:(n2 + 1) * 512], pp[:])
                else:
                    nc.vector.tensor_copy(o_sb[:, n2 * 512:(n2 + 1) * 512], pp[:])
            nc.sync.dma_start(out_p[i0:i0 + 128, :], o_sb[:])

        def rep_body(rep):
            pending = []        # i-tiles whose projection is deferred
            # HAM warm-up: keep the PE busy through the input-DMA window so the
            # projection matmuls run at 2.4GHz instead of the cold 1.2GHz.
            warm = ot_pool.tile([128, 128], F32, tag="ot", name="warm_ps")
            for _ in range(32):
                nc.tensor.matmul(warm[:], ident[:], ident[:], start=True, stop=True)
            xt_sb = xq_pool.tile([128, KQ, nq], FR, name="xt_sb")
            for k in range(KQ):
                nc.sync.dma_start(xt_sb[:, k, :], xT[k * 128:(k + 1) * 128, :])
            ct_jbs = []
            for jb in range(NJB):
                ct_jb = ck_pool.tile([128, KK, 512], FR, tag="ct", name=f"ct{jb}")
                for k in range(KK):
                    nc.sync.dma_start(
                        ct_jb[:, k, :],
                        cT[k * 128:(k + 1) * 128, jb * 512:(jb + 1) * 512])
                ct_jbs.append(ct_jb)

            # Q^T projection (needs all of xT)
            for n in range(nq // 512):
                ps = s_pool.tile([128, 512], F32, tag="s", name="q_ps")
                for k in range(KQ):
                    nc.tensor.matmul(ps[:], wq_sb[:, k, :],
                                     xt_sb[:, k, n * 512:(n + 1) * 512],
                                     start=(k == 0), stop=(k == KQ - 1))
                nc.vector.tensor_copy(qt_sb[:, n * 512:(n + 1) * 512], ps[:])

            def produce_jb(jb):
                """K^T/V for one 512-key block (consumes ct_jbs[jb])."""
                ps = s_pool.tile([128, 512], F32, tag="s", name="k_ps")
                for k in range(KK):
                    nc.tensor.matmul(ps[:], wk_sb[:, k, :], ct_jbs[jb][:, k, :],
                                     start=(k == 0), stop=(k == KK - 1))
                nc.vector.tensor_copy(kt_jbs[jb][:], ps[:])
                vt = ck_pool.tile([128, 512], FR, tag="vt", name=f"vt{jb}")
                ps = s_pool.tile([128, 512], F32, tag="s", name="v_ps")
                for k in range(KK):
                    nc.tensor.matmul(ps[:], wv_sb[:, k, :], ct_jbs[jb][:, k, :],
                                     start=(k == 0), stop=(k == KK - 1))
                nc.vector.tensor_copy(vt[:], ps[:])
                for t in range(4):
                    tp = s_pool.tile([128, 128], FR, tag="s", name="tr_ps")
                    nc.tensor.transpose(tp[:], vt[:, t * 128:(t + 1) * 128], ident[:])
                    nc.vector.tensor_copy(vaug_jbs[jb][:, t, 0:64], tp[:, 0:64])
                    nc.vector.tensor_copy(vaug_jbs[jb][:, t, 65:129], tp[:, 64:128])

            produce_jb(0)

            # ---------------- attention (+pipelined production/projection) ----
            for icp in range(NICP):
                ot_ps = [[ot_pool.tile([128, 512], F32, tag="ot", name=f"ot_{h}_{i}")
                          for i in range(2)] for h in range(HPC)]
                for jt in range(NJT):
                    jb, t = jt // 4, jt % 4
                    if icp == 0 and t == 0 and jb + 1 < NJB:
                        produce_jb(jb + 1)
                    s_ps = [s_pool.tile([128, 1024], F32, tag="s", name=f"s_{h}")
                            for h in range(HPC)]
                    for icc in range(2):
                        i0 = icp * 1024 + icc * 512
                        for h in range(HPC):   # alternate heads: PE row-group overlap
                            nc.tensor.matmul(
                                s_ps[h][:, icc * 512:(icc + 1) * 512],
                                kt_jbs[jb][h * 64:(h + 1) * 64, t * 128:(t + 1) * 128],
                                qt_sb[h * 64:(h + 1) * 64, i0:i0 + 512],
                                start=True, stop=True)
                    e_sb = [e_pool.tile([128, 1024], FR, tag="e", name=f"e_{h}")
                            for h in range(HPC)]
                    for h in range(HPC):
                        nc.scalar.activation(e_sb[h][:], s_ps[h][:], EXP, scale=SCALE)
                    first, last = (jt == 0), (jt == NJT - 1)
                    for icc in range(2):
                        esl = [e_sb[h][:, icc * 512:(icc + 1) * 512] for h in range(HPC)]
                        nc.tensor.matmul(ot_ps[0][icc][0:65, :],
                                         vaug_jbs[jb][:, t, 0:65], esl[0],
                                         start=first, stop=last)
                        nc.tensor.matmul(ot_ps[1][icc][0:65, :],
                                         vaug_jbs[jb][:, t, 65:130], esl[1],
                                         start=first, stop=last)
                        if icc == 1 and 2 <= jt < 2 + len(pending):
                            proj_tile(pending[jt - 2], tail=False)
                # normalize: O^T[:, i] /= denom[i]; head1 shifted to rows 64..127
                for icc in range(2):
                    i0 = icp * 1024 + icc * 512
                    r0 = r_pool.tile([65, 512], F32, tag="r0", bufs=1)
                    rb0 = r_pool.tile([64, 512], F32, tag="rb0", bufs=1)
                    nc.vector.reciprocal(r0[64:65, :], ot_ps[0][icc][64:65, :])
                    bcast64(r0[64:65, :], rb0[0:64, :], f"{rep}_{icp}_{icc}_0")
                    nc.vector.tensor_mul(ot_sb[0:64, i0:i0 + 512],
                                         ot_ps[0][icc][0:64, :], rb0[0:64, :])
                    r1 = r_pool.tile([65, 512], F32, tag="r1", bufs=1)
                    rb1 = r_pool.tile([64, 512], F32, tag="rb1", bufs=1)
                    nc.vector.reciprocal(r1[64:65, :], ot_ps[1][icc][64:65, :])
                    bcast64(r1[64:65, :], rb1[0:64, :], f"{rep}_{icp}_{icc}_1")
                    tmp = r_pool.tile([64, 512], FR, tag="tmp", bufs=1)
                    nc.vector.tensor_mul(tmp[0:64, :],
                                         ot_ps[1][icc][0:64, :], rb1[0:64, :])
                    nc.sync.dma_start(ot_sb[64:128, i0:i0 + 512], tmp[0:64, :])
                    if icp == NICP - 1:
                        # tail: project this icc's tiles once it is normalized
                        for tt in range(4):
                            proj_tile(icp * 8 + icc * 4 + tt, tail=True)
                if icp < NICP - 1:
                    pending = [icp * 8 + tt for tt in range(8)]

        if loop_iters > 1:
            with tc.For_i(0, loop_iters, 1, hint_engines=(mybir.EngineType.PE,)):
                rep_body(0)
        else:
            for rep in range(reps):
                rep_body(rep)

    nc.compile()
    return nc


_NC_CACHE = {}


def _get_nc():
    key = "full"
    if key not in _NC_CACHE:
        from concourse.bass_interp import get_hw_module
        nc = build_nc()
        nc.m = get_hw_module(nc.m)
        _NC_CACHE[key] = nc
    return _NC_CACHE[key]


def make_in_maps(x, context, Wq, Wk, Wv, Wo):
    """Host-side sharding: per-core input dicts."""
    x = np.asarray(x, np.float32)
    context = np.asarray(context, np.float32)
    Wq = np.asarray(Wq, np.float32)
    Wk = np.asarray(Wk, np.float32)
    Wv = np.asarray(Wv, np.float32)
    Wo = np.asarray(Wo, np.float32)
    xT = [np.ascontiguousarray(x[b].T) for b in range(B)]
    cT = [np.ascontiguousarray(context[b].T) for b in range(B)]
    in_maps = []
    for c in range(N_CORES):
        b, hp = c // 4, c % 4
        s0 = hp * IC
        in_maps.append({
            "xT": xT[b],
            "cT": cT[b],
            "wq": np.ascontiguousarray(
                Wq[:, s0:s0 + IC].reshape(CQ // 128, 128, IC).transpose(1, 0, 2)),
            "wk": np.ascontiguousarray(
                Wk[:, s0:s0 + IC].reshape(CK // 128, 128, IC).transpose(1, 0, 2)),
            "wv": np.ascontiguousarray(
                Wv[:, s0:s0 + IC].reshape(CK // 128, 128, IC).transpose(1, 0, 2)),
            "wo": np.ascontiguousarray(Wo[s0:s0 + IC, :]),
        })
    return in_maps


def gather_out(results, bo):
    out = np.zeros((B, NQ, CQ), np.float32)
    for c in range(N_CORES):
        out[c // 4] += results[c]["out_p"]
    out += np.asarray(bo, np.float32)
    return out


def kernel(x, context, Wq, Wk, Wv, Wo, bo):
    from concourse.bass_utils import run_bass_kernel_spmd
    nc = _get_nc()
    in_maps = make_in_maps(x, context, Wq, Wk, Wv, Wo)
    res = run_bass_kernel_spmd(nc, in_maps, core_ids=list(range(N_CORES)))
    return gather_out(res.results, bo)



# revision 5
# speedup vs baseline: 1.1112x; 1.1112x over previous
"""Trainium2 Bass kernel for CrossAttention (B=2, Nq=Nkv=2048, Cq=1024, Ck=768, H=8, D=64).

Sharding: batch*heads across 8 cores — core c handles batch c//4 and heads
{2*(c%4), 2*(c%4)+1} (a 128-wide slice of the inner dim I=512).  Wq/Wk/Wv are
column-sharded, Wo row-sharded; each core produces a partial (2048, 1024)
output and the host sums the 4 partials per batch (the "all-reduce").

All data is bf16 on the wire and in SBUF (PSUM accumulation in fp32): halves
HBM/DMA traffic vs fp32 while the PE runs at the same 1 col/cycle rate.

Per-core dataflow:
  xT (Cq x Nq) -> QT = Wq^T @ x       (128 x Nq)   [I_c on partitions]
  cT (Ck x Nkv), streamed in 512-key blocks:
      KT block = Wk^T @ ctx, VT block = Wv^T @ ctx, then PE-transpose VT
      into V j-tiles (the [V_h | 1] "vaug" layout).  Block production is
      software-pipelined into the first superchunk's attention loop.
  scores  S^T[j,i] = K_h Q_h^T  (transposed: j on partitions, 512-wide i)
  E = exp(SCALE * S^T)  via ScalarE straight out of PSUM (1024-wide reads)
  O^T[d,i] accumulated as matmul(lhsT=[V_h | ones], rhs=E) over j-tiles;
     the ones column yields the softmax denominators in the same pass.
  normalize O^T columns by 1/denom: reciprocal rows are broadcast down 64
     partitions with a rank-1 PE matmul (lhsT=ones row) instead of a DRAM
     roundtrip; head 1 partition-shifted into rows 64..127 via SBUF DMA.
  out_partial = O @ Wo_slice, pipelined into the next superchunk's loop.
DMA queues: inputs on SP, outputs + shifts on the Pool/GpSimd queue.
"""

import numpy as np
from contextlib import ExitStack

import concourse.bass as bass
import concourse.bacc as bacc
import concourse.mybir as mybir
import concourse.tile as tile
from concourse.masks import make_identity

F32 = mybir.dt.float32
FR = mybir.dt.float32r     # full-rate fp32 matmul mode on trn2
BF = mybir.dt.bfloat16
EXP = mybir.ActivationFunctionType.Exp

B, NQ, NKV, CQ, CK, H, D = 2, 2048, 2048, 1024, 768, 8, 64
HPC = 2                 # heads per core
IC = HPC * D            # 128: per-core slice of I = H*D
N_CORES = 8
SCALE = float(D) ** -0.5


def build_nc(nq=NQ, nkv=NKV, cq=CQ, ck=CK, num_devices=N_CORES, reps=1, loop_iters=1):
    """Build + compile the per-core SPMD Bass program."""
    assert nq % 1024 == 0 and nkv % 512 == 0 and cq % 512 == 0 and ck % 128 == 0
    KQ = cq // 128          # contraction tiles for the q projection
    KK = ck // 128          # contraction tiles for k/v projections
    NJT = nkv // 128        # key tiles
    NJB = nkv // 512        # key blocks (4 j-tiles each)
    NICP = nq // 1024       # 1024-wide query superchunks
    NCC = cq // 512         # output-column chunks

    nc = bacc.Bacc("TRN2", target_bir_lowering=False, debug=False,
                   enable_asserts=False, num_devices=num_devices)

    xT = nc.dram_tensor("xT", [128, KQ, nq], BF, kind="ExternalInput").ap()
    cT = nc.dram_tensor("cT", [128, KK, nkv], BF, kind="ExternalInput").ap()
    wq = nc.dram_tensor("wq", [128, KQ, 128], BF, kind="ExternalInput").ap()
    wk = nc.dram_tensor("wk", [128, KK, 128], BF, kind="ExternalInput").ap()
    wv = nc.dram_tensor("wv", [128, KK, 128], BF, kind="ExternalInput").ap()
    wo = nc.dram_tensor("wo", [128, cq], BF, kind="ExternalInput").ap()
    out_p = nc.dram_tensor("out_p", [nq, cq], BF, kind="ExternalOutput").ap()

    with tile.TileContext(nc) as tc, ExitStack() as ctx:
        const = ctx.enter_context(tc.tile_pool(name="const", bufs=1))
        wq_sb = const.tile([128, KQ, 128], BF)
        wk_sb = const.tile([128, KK, 128], BF)
        wv_sb = const.tile([128, KK, 128], BF)
        wo_sb = const.tile([128, cq], BF)
        ident_f = const.tile([128, 128], F32)
        ident = const.tile([128, 128], BF)
        ones_f = const.tile([128, 64], F32)
        ones_fr = const.tile([128, 64], FR)

        persist = ctx.enter_context(tc.tile_pool(name="persist", bufs=1))
        qt_sb = persist.tile([128, nq], BF)       # Q^T, I_c x Nq
        kt_jbs = [persist.tile([128, 512], BF, name=f"kt{jb}")
                  for jb in range(NJB)]           # K^T per key block
        vaug_jbs = [persist.tile([128, 4, 130], BF, name=f"vg{jb}")
                    for jb in range(NJB)]         # [V_h0 |1| V_h1 |1] per j-tile
        ot_sb = persist.tile([128, nq], BF)       # normalized O^T

        # PSUM pools: "s" = 2 x (128,1024) slots shared by scores / projections /
        # recip-broadcast psum; "ot" = 4 x 1-bank accumulator slots.
        s_pool = ctx.enter_context(tc.tile_pool(name="s", bufs=2, space="PSUM"))
        ot_pool = ctx.enter_context(tc.tile_pool(name="otp", bufs=4, space="PSUM"))
        e_pool = ctx.enter_context(tc.tile_pool(name="e", bufs=3))
        r_pool = ctx.enter_context(tc.tile_pool(name="r", bufs=1))
        o_pool = ctx.enter_context(tc.tile_pool(name="o", bufs=3))
        xq_pool = ctx.enter_context(tc.tile_pool(name="xq", bufs=1))
        ck_pool = ctx.enter_context(tc.tile_pool(name="ck", bufs=2))

        with tc.high_priority():
            nc.sync.dma_start(wq_sb[:], wq)
            nc.sync.dma_start(wk_sb[:], wk)
            nc.sync.dma_start(wv_sb[:], wv)
            nc.sync.dma_start(wo_sb[:], wo)
        make_identity(nc, ident_f[:])
        nc.vector.tensor_copy(ident[:], ident_f[:])
        nc.vector.memset(ones_f[:], 1.0)
        nc.vector.tensor_copy(ones_fr[:], ones_f[:])
        # softmax-denominator ones columns (static, written once)
        for jb in range(NJB):
            nc.vector.tensor_copy(vaug_jbs[jb][:, 0:4, 64:65], ones_f[:, 0:4])
            nc.vector.tensor_copy(vaug_jbs[jb][:, 0:4, 129:130], ones_f[:, 0:4])

        def proj_tile(gi, tail):
            """Project one 128-query tile of normalized O^T through Wo."""
            i0 = gi * 128
            o_sb = o_pool.tile([128, cq], BF, tag="o", name="o_sb")
            for n2 in range(NCC):
                pp = s_pool.tile([128, 512], F32, tag="s", name="pp")
                nc.tensor.matmul(pp[:], ot_sb[:, i0:i0 + 128],
                                 wo_sb[:, n2 * 512:(n2 + 1) * 512],
                                 start=True, stop=True)
                if tail and n2 % 2 == 1:
                    nc.scalar.copy(o_sb[:, n2 * 512:(n2 + 1) * 512], pp[:])
                else:
                    nc.vector.tensor_copy(o_sb[:, n2 * 512:(n2 + 1) * 512], pp[:])
            nc.gpsimd.dma_start(out_p[i0:i0 + 128, :], o_sb[:])

        def rep_body(rep):
            pending = []        # i-tiles whose projection is deferred
            # HAM warm-up: keep the PE busy through the input-DMA window so the
            # projection matmuls run at 2.4GHz instead of the cold 1.2GHz.
            warm = ot_pool.tile([128, 128], F32, tag="ot", name="warm_ps")
            for _ in range(16):
                nc.tensor.matmul(warm[:], ident[:], ident[:], start=True, stop=True)
            xt_sb = xq_pool.tile([128, KQ, nq], BF, name="xt_sb")
            nc.sync.dma_start(xt_sb[:], xT)
            ct_jbs = []
            for jb in range(NJB):
                ct_jb = ck_pool.tile([128, KK, 512], BF, tag="ct", name=f"ct{jb}")
                nc.sync.dma_start(ct_jb[:], cT[:, :, jb * 512:(jb + 1) * 512])
                ct_jbs.append(ct_jb)

            # Q^T projection (needs all of xT)
            for n in range(nq // 512):
                ps = s_pool.tile([128, 512], F32, tag="s", name="q_ps")
                for k in range(KQ):
                    nc.tensor.matmul(ps[:], wq_sb[:, k, :],
                                     xt_sb[:, k, n * 512:(n + 1) * 512],
                                     start=(k == 0), stop=(k == KQ - 1))
                nc.vector.tensor_copy(qt_sb[:, n * 512:(n + 1) * 512], ps[:])

            def produce_jb(jb):
                """K^T/V for one 512-key block (consumes ct_jbs[jb])."""
                ps = s_pool.tile([128, 512], F32, tag="s", name="k_ps")
                for k in range(KK):
                    nc.tensor.matmul(ps[:], wk_sb[:, k, :], ct_jbs[jb][:, k, :],
                                     start=(k == 0), stop=(k == KK - 1))
                nc.vector.tensor_copy(kt_jbs[jb][:], ps[:])
                vt = ck_pool.tile([128, 512], BF, tag="vt", name=f"vt{jb}")
                ps = s_pool.tile([128, 512], F32, tag="s", name="v_ps")
                for k in range(KK):
                    nc.tensor.matmul(ps[:], wv_sb[:, k, :], ct_jbs[jb][:, k, :],
                                     start=(k == 0), stop=(k == KK - 1))
                nc.vector.tensor_copy(vt[:], ps[:])
                for t in range(4):
                    tp = s_pool.tile([128, 128], BF, tag="s", name="tr_ps")
                    nc.tensor.transpose(tp[:], vt[:, t * 128:(t + 1) * 128], ident[:])
                    nc.vector.tensor_copy(vaug_jbs[jb][:, t, 0:64], tp[:, 0:64])
                    nc.vector.tensor_copy(vaug_jbs[jb][:, t, 65:129], tp[:, 64:128])

            produce_jb(0)

            # ---------------- attention (+pipelined production/projection) ----
            for icp in range(NICP):
                ot_ps = [[ot_pool.tile([128, 512], F32, tag="ot", name=f"ot_{h}_{i}")
                          for i in range(2)] for h in range(HPC)]
                for jt in range(NJT):
                    jb, t = jt // 4, jt % 4
                    if icp == 0 and t == 0 and jb + 1 < NJB:
                        produce_jb(jb + 1)
                    s_ps = [s_pool.tile([128, 1024], F32, tag="s", name=f"s_{h}")
                            for h in range(HPC)]
                    for icc in range(2):
                        i0 = icp * 1024 + icc * 512
                        for h in range(HPC):   # alternate heads: PE row-group overlap
                            nc.tensor.matmul(
                                s_ps[h][:, icc * 512:(icc + 1) * 512],
                                kt_jbs[jb][h * 64:(h + 1) * 64, t * 128:(t + 1) * 128],
                                qt_sb[h * 64:(h + 1) * 64, i0:i0 + 512],
                                start=True, stop=True)
                    e_sb = [e_pool.tile([128, 1024], BF, tag="e", name=f"e_{h}")
                            for h in range(HPC)]
                    for h in range(HPC):
                        nc.scalar.activation(e_sb[h][:], s_ps[h][:], EXP, scale=SCALE)
                    first, last = (jt == 0), (jt == NJT - 1)
                    for icc in range(2):
                        esl = [e_sb[h][:, icc * 512:(icc + 1) * 512] for h in range(HPC)]
                        nc.tensor.matmul(ot_ps[0][icc][0:65, :],
                                         vaug_jbs[jb][:, t, 0:65], esl[0],
                                         start=first, stop=last)
                        nc.tensor.matmul(ot_ps[1][icc][0:65, :],
                                         vaug_jbs[jb][:, t, 65:130], esl[1],
                                         start=first, stop=last)
                        if icc == 1 and 2 <= jt < 2 + len(pending):
                            proj_tile(pending[jt - 2], tail=False)
                # normalize: O^T[:, i] /= denom[i]; head1 shifted to rows 64..127.
                # Reciprocal rows are PE-broadcast down 64 partitions (rank-1
                # matmul with a ones row) — no DRAM roundtrip.
                for icc in range(2):
                    i0 = icp * 1024 + icc * 512
                    r_sb = r_pool.tile([65, 1024], FR, tag="r", bufs=2)
                    # float32r is full fp32 precision (PE rate mode) — the
                    # low-precision gate only special-cases exact float32.
                    with nc.allow_low_precision(reason="f32r == f32 precision"):
                        nc.vector.reciprocal(r_sb[64:65, 0:512],
                                             ot_ps[0][icc][64:65, :])
                        nc.vector.reciprocal(r_sb[64:65, 512:1024],
                                             ot_ps[1][icc][64:65, :])
                    rbp = s_pool.tile([128, 1024], F32, tag="s", name="rbp")
                    nc.tensor.matmul(rbp[0:64, 0:512], ones_fr[64:65, 0:64],
                                     r_sb[64:65, 0:512], start=True, stop=True)
                    nc.tensor.matmul(rbp[0:64, 512:1024], ones_fr[64:65, 0:64],
                                     r_sb[64:65, 512:1024], start=True, stop=True)
                    rb_sb = r_pool.tile([64, 1024], BF, tag="rb", bufs=2)
                    nc.vector.tensor_copy(rb_sb[:], rbp[0:64, :])
                    nc.vector.tensor_mul(ot_sb[0:64, i0:i0 + 512],
                                         ot_ps[0][icc][0:64, :], rb_sb[0:64, 0:512])
                    tmp = r_pool.tile([64, 512], BF, tag="tmp", bufs=2)
                    nc.vector.tensor_mul(tmp[0:64, :],
                                         ot_ps[1][icc][0:64, :], rb_sb[0:64, 512:1024])
                    nc.gpsimd.dma_start(ot_sb[64:128, i0:i0 + 512], tmp[0:64, :])
                    if icp == NICP - 1:
                        # tail: project this icc's tiles once it is normalized
                        for tt in range(4):
                            proj_tile(icp * 8 + icc * 4 + tt, tail=True)
                if icp < NICP - 1:
                    pending = [icp * 8 + tt for tt in range(8)]

        if loop_iters > 1:
            with tc.For_i(0, loop_iters, 1, hint_engines=(mybir.EngineType.PE,)):
                rep_body(0)
        else:
            for rep in range(reps):
                rep_body(rep)

    nc.compile()
    return nc


_NC_CACHE = {}


def _get_nc():
    key = "full"
    if key not in _NC_CACHE:
        from concourse.bass_interp import get_hw_module
        nc = build_nc()
        nc.m = get_hw_module(nc.m)
        _NC_CACHE[key] = nc
    return _NC_CACHE[key]


def make_in_maps(x, context, Wq, Wk, Wv, Wo):
    """Host-side sharding: per-core input dicts (bf16 on the wire)."""
    import ml_dtypes
    BF_NP = ml_dtypes.bfloat16
    x = np.asarray(x, np.float32)
    context = np.asarray(context, np.float32)
    Wq = np.asarray(Wq, np.float32)
    Wk = np.asarray(Wk, np.float32)
    Wv = np.asarray(Wv, np.float32)
    Wo = np.asarray(Wo, np.float32)
    # x[b].T reshaped to [128, KQ, Nq] so the device loads it in one DMA
    xT = [np.ascontiguousarray(
        x[b].T.reshape(CQ // 128, 128, NQ).transpose(1, 0, 2)).astype(BF_NP)
        for b in range(B)]
    cT = [np.ascontiguousarray(
        context[b].T.reshape(CK // 128, 128, NKV).transpose(1, 0, 2)).astype(BF_NP)
        for b in range(B)]
    in_maps = []
    for c in range(N_CORES):
        b, hp = c // 4, c % 4
        s0 = hp * IC
        in_maps.append({
            "xT": xT[b],
            "cT": cT[b],
            "wq": np.ascontiguousarray(
                Wq[:, s0:s0 + IC].reshape(CQ // 128, 128, IC).transpose(1, 0, 2)
            ).astype(BF_NP),
            "wk": np.ascontiguousarray(
                Wk[:, s0:s0 + IC].reshape(CK // 128, 128, IC).transpose(1, 0, 2)
            ).astype(BF_NP),
            "wv": np.ascontiguousarray(
                Wv[:, s0:s0 + IC].reshape(CK // 128, 128, IC).transpose(1, 0, 2)
            ).astype(BF_NP),
            "wo": np.ascontiguousarray(Wo[s0:s0 + IC, :]).astype(BF_NP),
        })
    return in_maps


def gather_out(results, bo):
    out = np.zeros((B, NQ, CQ), np.float32)
    for c in range(N_CORES):
        out[c // 4] += results[c]["out_p"].astype(np.float32)
    out += np.asarray(bo, np.float32)
    return out


def kernel(x, context, Wq, Wk, Wv, Wo, bo):
    from concourse.bass_utils import run_bass_kernel_spmd
    nc = _get_nc()
    in_maps = make_in_maps(x, context, Wq, Wk, Wv, Wo)
    res = run_bass_kernel_spmd(nc, in_maps, core_ids=list(range(N_CORES)))
    return gather_out(res.results, bo)


# revision 16
# speedup vs baseline: 1.3940x; 1.2545x over previous
"""Trainium2 Bass kernel for CrossAttention (B=2, Nq=Nkv=2048, Cq=1024, Ck=768, H=8, D=64).

Sharding: batch*heads across 8 cores — core c handles batch c//4 and heads
{2*(c%4), 2*(c%4)+1} (a 128-wide slice of the inner dim I=512).  Wq/Wk/Wv are
column-sharded, Wo row-sharded; each core produces a partial (2048, 1024)
output and the host sums the 4 partials per batch (the "all-reduce").

All data is bf16 on the wire and in SBUF (PSUM accumulation in fp32): halves
HBM/DMA traffic vs fp32 while the PE runs at the same 1 col/cycle rate.

Per-core dataflow:
  xT (Cq x Nq) -> QT = Wq^T @ x       (128 x Nq)   [I_c on partitions]
  cT (Ck x Nkv), streamed in 512-key blocks:
      KT block = Wk^T @ ctx, VT block = Wv^T @ ctx, then PE-transpose VT
      into V j-tiles (the [V_h | 1] "vaug" layout).  Block production is
      software-pipelined into the first superchunk's attention loop.
  scores  S^T[j,i] = K_h Q_h^T  (transposed: j on partitions, 512-wide i)
  E = exp(SCALE * S^T)  via ScalarE straight out of PSUM (1024-wide reads)
  O^T[d,i] accumulated as matmul(lhsT=[V_h | ones], rhs=E) over j-tiles;
     the ones column yields the softmax denominators in the same pass.
  normalize O^T columns by 1/denom: reciprocal rows are broadcast down 64
     partitions with a rank-1 PE matmul (lhsT=ones row) instead of a DRAM
     roundtrip; head 1 partition-shifted into rows 64..127 via SBUF DMA.
  out_partial = O @ Wo_slice, pipelined into the next superchunk's loop.
DMA queues: inputs on SP, outputs + shifts on the Pool/GpSimd queue.
"""

import numpy as np
from contextlib import ExitStack

import concourse.bass as bass
import concourse.bacc as bacc
import concourse.mybir as mybir
import concourse.tile as tile
from concourse.masks import make_identity

F32 = mybir.dt.float32
FR = mybir.dt.float32r     # full-rate fp32 matmul mode on trn2
BF = mybir.dt.bfloat16
EXP = mybir.ActivationFunctionType.Exp

B, NQ, NKV, CQ, CK, H, D = 2, 2048, 2048, 1024, 768, 8, 64
HPC = 2                 # heads per core
IC = HPC * D            # 128: per-core slice of I = H*D
N_CORES = 8
SCALE = float(D) ** -0.5


def build_nc(nq=NQ, nkv=NKV, cq=CQ, ck=CK, num_devices=N_CORES, reps=1, loop_iters=1):
    """Build + compile the per-core SPMD Bass program."""
    assert nq % 1024 == 0 and nkv % 512 == 0 and cq % 512 == 0 and ck % 128 == 0
    KQ = cq // 128          # contraction tiles for the q projection
    KK = ck // 128          # contraction tiles for k/v projections
    NJT = nkv // 128        # key tiles
    NJB = nkv // 512        # key blocks (4 j-tiles each)
    NICP = nq // 1024       # 1024-wide query superchunks
    NCC = cq // 512         # output-column chunks

    nc = bacc.Bacc("TRN2", target_bir_lowering=False, debug=False,
                   enable_asserts=False, num_devices=num_devices)

    xT = nc.dram_tensor("xT", [128, KQ, nq], BF, kind="ExternalInput").ap()
    cT = nc.dram_tensor("cT", [128, KK, nkv], BF, kind="ExternalInput").ap()
    wq = nc.dram_tensor("wq", [128, KQ, 128], BF, kind="ExternalInput").ap()
    wk = nc.dram_tensor("wk", [128, KK, 128], BF, kind="ExternalInput").ap()
    wv = nc.dram_tensor("wv", [128, KK, 128], BF, kind="ExternalInput").ap()
    wo = nc.dram_tensor("wo", [128, cq], BF, kind="ExternalInput").ap()
    out_p = nc.dram_tensor("out_p", [nq, cq], BF, kind="ExternalOutput").ap()

    with tile.TileContext(nc) as tc, ExitStack() as ctx:
        const = ctx.enter_context(tc.tile_pool(name="const", bufs=1))
        wq_sb = const.tile([128, KQ, 128], BF)
        wk_sb = const.tile([128, KK, 128], BF)
        wv_sb = const.tile([128, KK, 128], BF)
        wo_sb = const.tile([128, cq], BF)
        ident_f = const.tile([128, 128], F32)
        ident = const.tile([128, 128], BF)
        ones_f = const.tile([128, 64], F32)
        ones_fr = const.tile([128, 64], FR)

        persist = ctx.enter_context(tc.tile_pool(name="persist", bufs=1))
        qt_sb = persist.tile([128, nq], BF)       # Q^T, I_c x Nq
        kt_jbs = [persist.tile([128, 512], BF, name=f"kt{jb}")
                  for jb in range(NJB)]           # K^T per key block
        vaug_jbs = [persist.tile([128, 4, 130], BF, name=f"vg{jb}")
                    for jb in range(NJB)]         # [V_h0 |1| V_h1 |1] per j-tile
        ot_sb = persist.tile([128, nq], BF)       # normalized O^T

        # PSUM pools: "s" = 2 x (128,1024) slots shared by scores / projections /
        # recip-broadcast psum; "ot" = 4 x 1-bank accumulator slots.
        s_pool = ctx.enter_context(tc.tile_pool(name="s", bufs=2, space="PSUM"))
        ot_pool = ctx.enter_context(tc.tile_pool(name="otp", bufs=4, space="PSUM"))
        e_pool = ctx.enter_context(tc.tile_pool(name="e", bufs=6))
        r_pool = ctx.enter_context(tc.tile_pool(name="r", bufs=1))
        o_pool = ctx.enter_context(tc.tile_pool(name="o", bufs=4))
        xq_pool = ctx.enter_context(tc.tile_pool(name="xq", bufs=2))
        ck_pool = ctx.enter_context(tc.tile_pool(name="ck", bufs=3))

        with tc.high_priority():
            nc.sync.dma_start(wq_sb[:], wq)
            nc.sync.dma_start(wk_sb[:], wk)
            nc.sync.dma_start(wv_sb[:], wv)
            nc.sync.dma_start(wo_sb[:], wo)
        make_identity(nc, ident_f[:])
        nc.vector.tensor_copy(ident[:], ident_f[:])
        nc.vector.memset(ones_f[:], 1.0)
        nc.vector.tensor_copy(ones_fr[:], ones_f[:])
        # softmax-denominator ones columns (static, written once)
        for jb in range(NJB):
            nc.vector.tensor_copy(vaug_jbs[jb][:, 0:4, 64:65], ones_f[:, 0:4])
            nc.vector.tensor_copy(vaug_jbs[jb][:, 0:4, 129:130], ones_f[:, 0:4])

        def proj_tile(gi, tail):
            """Project one 128-query tile of normalized O^T through Wo."""
            i0 = gi * 128
            o_sb = o_pool.tile([128, cq], BF, tag="o", name="o_sb")
            pp = s_pool.tile([128, 1024], F32, tag="s", name="pp")
            for n2 in range(NCC):
                nc.tensor.matmul(pp[:, n2 * 512:(n2 + 1) * 512],
                                 ot_sb[:, i0:i0 + 128],
                                 wo_sb[:, n2 * 512:(n2 + 1) * 512],
                                 start=True, stop=True)
            if tail:
                nc.vector.tensor_copy(o_sb[:, 0:512], pp[:, 0:512])
                nc.scalar.copy(o_sb[:, 512:1024], pp[:, 512:1024])
            else:
                nc.vector.tensor_copy(o_sb[:], pp[:])
            nc.gpsimd.dma_start(out_p[i0:i0 + 128, :], o_sb[:])

        def rep_dmas():
            """Issue this rep's input DMAs (SP queue); prefetched one rep ahead."""
            xt_sb = xq_pool.tile([128, KQ, nq], BF, name="xt_sb")
            nc.sync.dma_start(xt_sb[:], xT)
            ct_jbs = []
            for jb in range(NJB):
                ct_jb = ck_pool.tile([128, KK, 512], BF, tag="ct", name=f"ct{jb}")
                nc.sync.dma_start(ct_jb[:], cT[:, :, jb * 512:(jb + 1) * 512])
                ct_jbs.append(ct_jb)
            return xt_sb, ct_jbs

        def produce_steps(jb, ct_jbs):
            """K^T/V for one 512-key block as 4 sub-steps, interleaved between
            key-tiles so the PE load stays smooth (no ScalarE starvation)."""
            state = {}

            def step_k():
                ps = s_pool.tile([128, 512], F32, tag="s", name="k_ps")
                for k in range(KK):
                    nc.tensor.matmul(ps[:], wk_sb[:, k, :], ct_jbs[jb][:, k, :],
                                     start=(k == 0), stop=(k == KK - 1))
                nc.vector.tensor_copy(kt_jbs[jb][:], ps[:])

            def step_v():
                vt = ck_pool.tile([128, 512], BF, tag="vt", name=f"vt{jb}")
                ps = s_pool.tile([128, 512], F32, tag="s", name="v_ps")
                for k in range(KK):
                    nc.tensor.matmul(ps[:], wv_sb[:, k, :], ct_jbs[jb][:, k, :],
                                     start=(k == 0), stop=(k == KK - 1))
                nc.vector.tensor_copy(vt[:], ps[:])
                state["vt"] = vt

            def step_tr(ts):
                def run():
                    vt = state["vt"]
                    for t in ts:
                        tp = s_pool.tile([128, 128], BF, tag="s", name="tr_ps")
                        nc.tensor.transpose(tp[:], vt[:, t * 128:(t + 1) * 128],
                                            ident[:])
                        nc.vector.tensor_copy(vaug_jbs[jb][:, t, 0:64], tp[:, 0:64])
                        nc.vector.tensor_copy(vaug_jbs[jb][:, t, 65:129],
                                              tp[:, 64:128])
                return run

            return [step_k, step_v, step_tr((0, 1)), step_tr((2, 3))]

        def compute_head(xt_sb, ct_jbs, cold):
            """Q^T projection + first key block (PE work for a fresh rep)."""
            if cold:
                # HAM warm-up: keep the PE busy through the input-DMA window so
                # the projection matmuls run at 2.4GHz instead of cold 1.2GHz.
                warm = ot_pool.tile([128, 128], F32, tag="ot", name="warm_ps")
                for _ in range(16):
                    nc.tensor.matmul(warm[:], ident[:], ident[:],
                                     start=True, stop=True)
            for n in range(nq // 512):
                ps = s_pool.tile([128, 512], F32, tag="s", name="q_ps")
                for k in range(KQ):
                    nc.tensor.matmul(ps[:], wq_sb[:, k, :],
                                     xt_sb[:, k, n * 512:(n + 1) * 512],
                                     start=(k == 0), stop=(k == KQ - 1))
                nc.vector.tensor_copy(qt_sb[:, n * 512:(n + 1) * 512], ps[:])
            for st in produce_steps(0, ct_jbs):
                st()

        def rep_attention(ct_jbs, next_inputs):
            """Attention + projections for one rep.  If next_inputs is set,
            the NEXT rep's Q projection + first key block are emitted into this
            rep's tail (while normalize runs on DVE) — PE never idles there."""
            pending = []        # i-tiles whose projection is deferred
            # AV for key-tile jt-2 is issued during key-tile jt's scores/exp:
            # the PE never waits on the exp chain (ScalarE becomes the pacer).
            for icp in range(NICP):
                ot_ps = [[ot_pool.tile([128, 512], F32, tag="ot", name=f"ot_{h}_{i}")
                          for i in range(2)] for h in range(HPC)]

                def do_av(pj, e_prev):
                    pjb, pt = pj // 4, pj % 4
                    first, last = (pj == 0), (pj == NJT - 1)
                    for icc in range(2):
                        esl = [e_prev[h][:, icc * 512:(icc + 1) * 512]
                               for h in range(HPC)]
                        nc.tensor.matmul(ot_ps[0][icc][0:65, :],
                                         vaug_jbs[pjb][:, pt, 0:65], esl[0],
                                         start=first, stop=last)
                        nc.tensor.matmul(ot_ps[1][icc][0:65, :],
                                         vaug_jbs[pjb][:, pt, 65:130], esl[1],
                                         start=first, stop=last)
                        if icc == 1 and 2 <= pj < 2 + len(pending):
                            proj_tile(pending[pj - 2], tail=False)

                av_q = []      # (key-tile, E) pairs awaiting AV, depth 2
                steps = None   # pending produce sub-steps for the next block
                for jt in range(NJT):
                    jb, t = jt // 4, jt % 4
                    if icp == 0 and jb + 1 < NJB:
                        if t == 0:
                            steps = produce_steps(jb + 1, ct_jbs)
                        steps[t]()
                    s_ps = [s_pool.tile([128, 1024], F32, tag="s", name=f"s_{h}")
                            for h in range(HPC)]
                    for icc in range(2):
                        i0 = icp * 1024 + icc * 512
                        for h in range(HPC):   # alternate heads: PE row-group overlap
                            nc.tensor.matmul(
                                s_ps[h][:, icc * 512:(icc + 1) * 512],
                                kt_jbs[jb][h * 64:(h + 1) * 64, t * 128:(t + 1) * 128],
                                qt_sb[h * 64:(h + 1) * 64, i0:i0 + 512],
                                start=True, stop=True)
                    e_sb = [e_pool.tile([128, 1024], BF, tag="e", name=f"e_{h}")
                            for h in range(HPC)]
                    for h in range(HPC):
                        nc.scalar.activation(e_sb[h][:], s_ps[h][:], EXP, scale=SCALE)
                    av_q.append((jt, e_sb))
                    if len(av_q) > 2:
                        do_av(*av_q.pop(0))
                for pj, pe in av_q:
                    do_av(pj, pe)
                # normalize: O^T[:, i] /= denom[i]; head1 shifted to rows 64..127.
                # Reciprocal rows are PE-broadcast down 64 partitions (rank-1
                # matmul with a ones row) — no DRAM roundtrip.
                for icc in range(2):
                    i0 = icp * 1024 + icc * 512
                    r_sb = r_pool.tile([65, 1024], FR, tag="r", bufs=2)
                    # float32r is full fp32 precision (PE rate mode) — the
                    # low-precision gate only special-cases exact float32.
                    with nc.allow_low_precision(reason="f32r == f32 precision"):
                        nc.vector.reciprocal(r_sb[64:65, 0:512],
                                             ot_ps[0][icc][64:65, :])
                        nc.vector.reciprocal(r_sb[64:65, 512:1024],
                                             ot_ps[1][icc][64:65, :])
                    rbp = s_pool.tile([128, 1024], F32, tag="s", name="rbp")
                    nc.tensor.matmul(rbp[0:64, 0:512], ones_fr[64:65, 0:64],
                                     r_sb[64:65, 0:512], start=True, stop=True)
                    nc.tensor.matmul(rbp[0:64, 512:1024], ones_fr[64:65, 0:64],
                                     r_sb[64:65, 512:1024], start=True, stop=True)
                    rb_sb = r_pool.tile([64, 1024], BF, tag="rb", bufs=2)
                    nc.vector.tensor_copy(rb_sb[:], rbp[0:64, :])
                    nc.vector.tensor_mul(ot_sb[0:64, i0:i0 + 512],
                                         ot_ps[0][icc][0:64, :], rb_sb[0:64, 0:512])
                    tmp = r_pool.tile([64, 512], BF, tag="tmp", bufs=2)
                    nc.vector.tensor_mul(tmp[0:64, :],
                                         ot_ps[1][icc][0:64, :], rb_sb[0:64, 512:1024])
                    nc.gpsimd.dma_start(ot_sb[64:128, i0:i0 + 512], tmp[0:64, :])
                    if icp == NICP - 1 and icc == 0 and next_inputs is not None:
                        # next rep's Qproj/first block fills the PE while this
                        # rep's normalize + tail projections run on DVE/ScalarE
                        compute_head(*next_inputs, cold=False)
                    if icp == NICP - 1:
                        # tail: project this icc's tiles once it is normalized
                        for tt in range(4):
                            proj_tile(icp * 8 + icc * 4 + tt, tail=True)
                if icp < NICP - 1:
                    pending = [icp * 8 + tt for tt in range(8)]

        def run_reps(n_reps):
            """Chain n_reps: inputs prefetched a rep ahead, heads pipelined
            into the previous rep's tail."""
            cur = rep_dmas()
            compute_head(*cur, cold=True)
            for u in range(n_reps):
                nxt = rep_dmas() if u + 1 < n_reps else None
                rep_attention(cur[1], nxt)
                if nxt is not None:
                    cur = nxt

        if loop_iters > 1:
            unroll = 2 if loop_iters % 2 == 0 else 1
            with tc.For_i(0, loop_iters // unroll, 1,
                          hint_engines=(mybir.EngineType.PE,)):
                run_reps(unroll)
        else:
            for rep in range(reps):
                run_reps(1)

    nc.compile()
    return nc


_NC_CACHE = {}


def _get_nc():
    key = "full"
    if key not in _NC_CACHE:
        from concourse.bass_interp import get_hw_module
        nc = build_nc()
        nc.m = get_hw_module(nc.m)
        _NC_CACHE[key] = nc
    return _NC_CACHE[key]


def make_in_maps(x, context, Wq, Wk, Wv, Wo):
    """Host-side sharding: per-core input dicts (bf16 on the wire)."""
    import ml_dtypes
    BF_NP = ml_dtypes.bfloat16
    x = np.asarray(x, np.float32)
    context = np.asarray(context, np.float32)
    Wq = np.asarray(Wq, np.float32)
    Wk = np.asarray(Wk, np.float32)
    Wv = np.asarray(Wv, np.float32)
    Wo = np.asarray(Wo, np.float32)
    # x[b].T reshaped to [128, KQ, Nq] so the device loads it in one DMA
    xT = [np.ascontiguousarray(
        x[b].T.reshape(CQ // 128, 128, NQ).transpose(1, 0, 2)).astype(BF_NP)
        for b in range(B)]
    cT = [np.ascontiguousarray(
        context[b].T.reshape(CK // 128, 128, NKV).transpose(1, 0, 2)).astype(BF_NP)
        for b in range(B)]
    in_maps = []
    for c in range(N_CORES):
        b, hp = c // 4, c % 4
        s0 = hp * IC
        in_maps.append({
            "xT": xT[b],
            "cT": cT[b],
            "wq": np.ascontiguousarray(
                Wq[:, s0:s0 + IC].reshape(CQ // 128, 128, IC).transpose(1, 0, 2)
            ).astype(BF_NP),
            "wk": np.ascontiguousarray(
                Wk[:, s0:s0 + IC].reshape(CK // 128, 128, IC).transpose(1, 0, 2)
            ).astype(BF_NP),
            "wv": np.ascontiguousarray(
                Wv[:, s0:s0 + IC].reshape(CK // 128, 128, IC).transpose(1, 0, 2)
            ).astype(BF_NP),
            "wo": np.ascontiguousarray(Wo[s0:s0 + IC, :]).astype(BF_NP),
        })
    return in_maps


def gather_out(results, bo):
    out = np.zeros((B, NQ, CQ), np.float32)
    for c in range(N_CORES):
        out[c // 4] += results[c]["out_p"].astype(np.float32)
    out += np.asarray(bo, np.float32)
    return out


def kernel(x, context, Wq, Wk, Wv, Wo, bo):
    from concourse.bass_utils import run_bass_kernel_spmd
    nc = _get_nc()
    in_maps = make_in_maps(x, context, Wq, Wk, Wv, Wo)
    res = run_bass_kernel_spmd(nc, in_maps, core_ids=list(range(N_CORES)))
    return gather_out(res.results, bo)
